# revision 9
# baseline (speedup 1.0000x reference)
"""Causal self-attention (B=2, T=2048, C=1024, H=16) on 8 TRN2 NeuronCores.

Sharding: 2 batches x 4 head-groups (4 heads each). Each core computes
qkv projection for its batch/head-slice, causal attention, and a partial
c_proj contribution; the host sums partials and adds b_proj.

Device layout (per core):
  xT  [C=1024, T=2048]  x[b] transposed (host-side)
  wT  [C=1024, 768]     w_attn rows for this head slice, transposed: cols
                        [0:256]=Q feats, [256:512]=K, [512:768]=V
  bqkv [768]            matching bias slice
  wpT [256, 1024]       w_proj columns for this head slice, transposed
  out yT [1024, 2048]   partial (c_proj output)^T, to be summed over the
                        4 cores of the batch on host

All matmuls run as float32r (full-rate fp32 on the PE, ~1.5e-4 rel err).
P = exp(scores) and V are bf16 for the attention*V matmul.
Causal structure: scores computed in S^T [key, query] orientation per
128-key-block x 512-query-chunk rectangles, only blocks intersecting the
causal triangle; diagonal blocks masked via memset + a precomputed
staircase mask. Softmax denominators come free from a ones-column
appended to V (row 64 of the AV psum accumulator).
"""
import sys
import contextlib

sys.path.insert(0, "/opt/trn_rl_repo")

import numpy as np
import ml_dtypes

import concourse.bass as bass
import concourse.mybir as mybir
import concourse.tile as tile
from concourse import bacc
from concourse.bass_utils import run_bass_kernel_spmd

B, T, C, H = 2, 2048, 1024, 16
HD = 64
N_CORES = 8
HPC = 4          # heads per core
FPC = HPC * HD   # features per core = 256
QCH = 512        # query chunk
NQC = T // QCH   # 4
NKB = T // 128   # 16 k blocks
NCC = C // 128   # 8 contraction chunks
NTC = T // 512   # 4 token chunks

F32 = mybir.dt.float32
F32R = mybir.dt.float32r
BF16 = mybir.dt.bfloat16

_CACHE: dict = {}


def _build():
    if "nc" in _CACHE:
        return _CACHE["nc"]
    nc = bacc.Bacc(None, target_bir_lowering=False, debug=False)

    xT_d = nc.dram_tensor("xT", [C, T], F32R, kind="ExternalInput").ap()
    wT_d = nc.dram_tensor("wT", [C, 3 * FPC], F32R, kind="ExternalInput").ap()
    bqkv_d = nc.dram_tensor("bqkv", [3 * FPC], F32, kind="ExternalInput").ap()
    wpT_d = nc.dram_tensor("wpT", [FPC, C], F32R, kind="ExternalInput").ap()
    yT_d = nc.dram_tensor("yT", [C, T], F32, kind="ExternalOutput").ap()

    stair_np = (np.tril(np.ones((128, 128), dtype=np.float32)).T).astype(
        ml_dtypes.bfloat16
    )  # stair[i, j] = 1 if i <= j else 0
    stair_d = nc.inline_tensor(stair_np, name="stair").ap()

    with tile.TileContext(nc) as tc:
        with contextlib.ExitStack() as ctx:
            consts = ctx.enter_context(tc.tile_pool(name="consts", bufs=1))
            xpool = ctx.enter_context(tc.tile_pool(name="x", bufs=1))
            qkpool = ctx.enter_context(tc.tile_pool(name="qk", bufs=1))
            vpool = ctx.enter_context(tc.tile_pool(name="v", bufs=1))
            ppool = ctx.enter_context(tc.tile_pool(name="p", bufs=2))
            ypool = ctx.enter_context(tc.tile_pool(name="y", bufs=2))
            opool = ctx.enter_context(tc.tile_pool(name="o", bufs=3))
            rpool = ctx.enter_context(tc.tile_pool(name="r", bufs=2))
            qkv_ps = ctx.enter_context(tc.tile_pool(name="qkv_ps", bufs=2, space="PSUM"))
            v_ps = ctx.enter_context(tc.tile_pool(name="v_ps", bufs=1, space="PSUM"))
            s_ps = ctx.enter_context(tc.tile_pool(name="s_ps", bufs=2, space="PSUM"))
            y_ps = ctx.enter_context(tc.tile_pool(name="y_ps", bufs=1, space="PSUM"))
            o_ps = ctx.enter_context(tc.tile_pool(name="o_ps", bufs=2, space="PSUM"))

            # ---- constants / weights ----
            wt = {}
            for cc in range(NCC):
                wt[cc] = consts.tile([128, 3 * FPC], F32R, tag=f"w{cc}", name=f"w{cc}")
                nc.sync.dma_start(out=wt[cc][:], in_=wT_d[cc * 128:(cc + 1) * 128, :])
            wp = {}
            for hc in range(2):
                wp[hc] = consts.tile([128, C], F32R, tag=f"wp{hc}", name=f"wp{hc}")
                nc.sync.dma_start(out=wp[hc][:], in_=wpT_d[hc * 128:(hc + 1) * 128, :])
            qk_bias = {}
            for fb in range(4):
                qk_bias[fb] = consts.tile([128, 1], F32, tag=f"qkb{fb}", name=f"qkb{fb}")
                nc.sync.dma_start(
                    out=qk_bias[fb][:],
                    in_=bqkv_d[fb * 128:(fb + 1) * 128].unsqueeze(-1),
                )
            v_bias = consts.tile([128, FPC], F32, tag="vbias")
            nc.sync.dma_start(
                out=v_bias[:], in_=bqkv_d[512:768].partition_broadcast(128)
            )
            stair = consts.tile([128, 128], BF16, tag="stair")
            nc.sync.dma_start(out=stair[:], in_=stair_d[:])

            # ---- x load (token-chunk-major so compute can start early) ----
            xt = {}
            for tc_i in range(NTC):
                for cc in range(NCC):
                    xt[(cc, tc_i)] = xpool.tile([128, 512], F32R, tag=f"x{cc}_{tc_i}", name=f"x{cc}_{tc_i}")
                    nc.sync.dma_start(
                        out=xt[(cc, tc_i)][:],
                        in_=xT_d[cc * 128:(cc + 1) * 128, tc_i * 512:(tc_i + 1) * 512],
                    )

            # ---- QKV projection ----
            # feature-major Q^T, K^T: qk[(fb, tc)] [128, 512], fb 0..1 = Q
            # (heads 0-1, 2-3), fb 2..3 = K
            qk = {}
            vt_by_tb = {}
            for tc_i in range(NTC):
                for fb in range(4):
                    ps = qkv_ps.tile([128, 512], F32, tag="qkvps")
                    for cc in range(NCC):
                        nc.tensor.matmul(
                            ps[:],
                            wt[cc][:, fb * 128:(fb + 1) * 128],
                            xt[(cc, tc_i)][:],
                            start=(cc == 0),
                            stop=(cc == NCC - 1),
                        )
                    qk[(fb, tc_i)] = qkpool.tile([128, 512], F32R, tag=f"qk{fb}_{tc_i}", name=f"qk{fb}_{tc_i}")
                    nc.vector.tensor_scalar_add(qk[(fb, tc_i)][:], ps[:], qk_bias[fb][:])
                # token-major V_ext tiles [128 tokens, 4 heads, 66] (64 V cols,
                # col 64 = ones for the softmax denominator, col 65 pad)
                for tb in range(tc_i * 4, tc_i * 4 + 4):
                    ps = v_ps.tile([128, FPC], F32, tag="vps")
                    for cc in range(NCC):
                        nc.tensor.matmul(
                            ps[:],
                            xt[(cc, tc_i)][:, (tb % 4) * 128:(tb % 4 + 1) * 128],
                            wt[cc][:, 512:768],
                            start=(cc == 0),
                            stop=(cc == NCC - 1),
                        )
                    vt = vpool.tile([128, HPC, 66], BF16, tag=f"v{tb}")
                    nc.vector.tensor_add(
                        vt[:, :, 0:64],
                        ps[:].rearrange("p (h d) -> p h d", h=HPC),
                        v_bias[:].rearrange("p (h d) -> p h d", h=HPC),
                    )
                    nc.vector.memset(vt[:, :, 64:65], 1.0)
                    vt_by_tb[tb] = vt

            # ---- attention + proj, per query chunk ----
            for qc in range(NQC):
                yT_pair = {}
                for hc in range(2):
                    yT_pair[hc] = ypool.tile([128, 512], F32R, tag=f"yp{hc}", name=f"yp{hc}_{qc}")
                kmax = 4 * (qc + 1)
                for h in range(HPC):
                    fbq, hb = h // 2, h % 2
                    prow = slice(hb * 64, hb * 64 + 64)
                    p_tiles = {}
                    for kb in range(kmax):
                        sp = s_ps.tile([128, 512], F32, tag="sps")
                        nc.tensor.matmul(
                            sp[:],
                            qk[(2 + fbq, kb // 4)][prow, (kb % 4) * 128:(kb % 4 + 1) * 128],
                            qk[(fbq, qc)][prow, :],
                            start=True,
                            stop=True,
                        )
                        pt = ppool.tile([128, 512], BF16, tag=f"p{kb}")
                        g = kb * 128 - qc * 512
                        if g <= -128:
                            nc.scalar.activation(
                                pt[:], sp[:], mybir.ActivationFunctionType.Exp,
                                scale=0.125,
                            )
                        else:
                            if g > 0:
                                nc.gpsimd.memset(pt[:, 0:g], 0.0)
                            nc.scalar.activation(
                                pt[:, g:512], sp[:, g:512],
                                mybir.ActivationFunctionType.Exp, scale=0.125,
                            )
                            nc.vector.tensor_mul(
                                pt[:, g:g + 127], pt[:, g:g + 127], stair[:, 0:127]
                            )
                        p_tiles[kb] = pt
                    yp = y_ps.tile([65, 512], F32, tag="yps")
                    for kb in range(kmax):
                        nc.tensor.matmul(
                            yp[:],
                            vt_by_tb[kb][:, h, 0:65],
                            p_tiles[kb][:],
                            start=(kb == 0),
                            stop=(kb == kmax - 1),
                        )
                    recip = rpool.tile([1, 512], F32, tag="recip")
                    nc.vector.reciprocal(recip[:], yp[64:65, :])
                    recip_b = rpool.tile([64, 512], F32, tag="recip_b")
                    nc.gpsimd.partition_broadcast(recip_b[:], recip[:])
                    nc.vector.tensor_mul(
                        yT_pair[h // 2][prow, :], yp[0:64, :], recip_b[:]
                    )
                # c_proj partial for this query chunk
                for ob in range(8):
                    op = o_ps.tile([128, 512], F32, tag="ops")
                    for hc in range(2):
                        nc.tensor.matmul(
                            op[:],
                            wp[hc][:, ob * 128:(ob + 1) * 128],
                            yT_pair[hc][:],
                            start=(hc == 0),
                            stop=(hc == 1),
                        )
                    ot = opool.tile([128, 512], F32, tag="ot")
                    nc.scalar.copy(ot[:], op[:])
                    nc.sync.dma_start(
                        out=yT_d[ob * 128:(ob + 1) * 128, qc * 512:(qc + 1) * 512],
                        in_=ot[:],
                    )

    nc.compile()
    _CACHE["nc"] = nc
    return nc


def kernel(x, w_attn, b_attn, w_proj, b_proj):
    nc = _build()
    in_maps = []
    for core in range(N_CORES):
        b, s = core // 4, core % 4
        f0 = FPC * s
        xT = np.ascontiguousarray(x[b].T)
        wT = np.ascontiguousarray(
            np.concatenate(
                [
                    w_attn[f0:f0 + FPC],
                    w_attn[C + f0:C + f0 + FPC],
                    w_attn[2 * C + f0:2 * C + f0 + FPC],
                ],
                axis=0,
            ).T
        )
        bqkv = np.ascontiguousarray(
            np.concatenate(
                [
                    b_attn[f0:f0 + FPC],
                    b_attn[C + f0:C + f0 + FPC],
                    b_attn[2 * C + f0:2 * C + f0 + FPC],
                ]
            )
        )
        wpT = np.ascontiguousarray(w_proj[:, f0:f0 + FPC].T)
        in_maps.append({"xT": xT, "wT": wT, "bqkv": bqkv, "wpT": wpT})

    res = run_bass_kernel_spmd(nc, in_maps, list(range(N_CORES)))
    out = np.empty((B, T, C), dtype=np.float32)
    for b in range(B):
        acc = res.results[4 * b]["yT"].astype(np.float32)
        for s in range(1, 4):
            acc = acc + res.results[4 * b + s]["yT"]
        out[b] = acc.T + b_proj[None, :]
    return out


# revision 13
# speedup vs baseline: 1.3995x; 1.3995x over previous
"""Causal self-attention (B=2, T=2048, C=1024, H=16) on 8 TRN2 NeuronCores.

Sharding: 2 batches x 4 head-groups (4 heads each). Each core computes
qkv projection for its batch/head-slice, causal attention, and a partial
c_proj contribution; the host sums partials and adds b_proj.

Device layout (per core):
  xT  [C=1024, T=2048]  x[b] transposed (host-side)
  wT  [C=1024, 768]     w_attn rows for this head slice, transposed: cols
                        [0:256]=Q feats, [256:512]=K, [512:768]=V
  bqkv [768]            matching bias slice
  wpT [256, 1024]       w_proj columns for this head slice, transposed
  out yT [1024, 2048]   partial (c_proj output)^T, to be summed over the
                        4 cores of the batch on host

All matmuls run as float32r (full-rate fp32 on the PE, ~1.5e-4 rel err).
P = exp(scores) and V are bf16 for the attention*V matmul.
Causal structure: scores computed in S^T [key, query] orientation per
128-key-block x 512-query-chunk rectangles, only blocks intersecting the
causal triangle; diagonal blocks masked via memset + a precomputed
staircase mask. Softmax denominators come free from a ones-column
appended to V (row 64 of the AV psum accumulator).
"""
import sys
import contextlib

sys.path.insert(0, "/opt/trn_rl_repo")

import numpy as np
import ml_dtypes

import concourse.bass as bass
import concourse.mybir as mybir
import concourse.tile as tile
from concourse import bacc
from concourse.bass_utils import run_bass_kernel_spmd

B, T, C, H = 2, 2048, 1024, 16
HD = 64
N_CORES = 8
HPC = 4          # heads per core
FPC = HPC * HD   # features per core = 256
QCH = 512        # query chunk
NQC = T // QCH   # 4
NKB = T // 128   # 16 k blocks
NCC = C // 128   # 8 contraction chunks
NTC = T // 512   # 4 token chunks

F32 = mybir.dt.float32
F32R = mybir.dt.float32r
BF16 = mybir.dt.bfloat16

_CACHE: dict = {}


def _build():
    if "nc" in _CACHE:
        return _CACHE["nc"]
    nc = bacc.Bacc(None, target_bir_lowering=False, debug=False)

    xT_d = nc.dram_tensor("xT", [C, T], F32R, kind="ExternalInput").ap()
    wT_d = nc.dram_tensor("wT", [C, 3 * FPC], F32R, kind="ExternalInput").ap()
    bqkv_d = nc.dram_tensor("bqkv", [3 * FPC], F32, kind="ExternalInput").ap()
    wpT_d = nc.dram_tensor("wpT", [FPC, C], F32R, kind="ExternalInput").ap()
    yT_d = nc.dram_tensor("yT", [C, T], F32, kind="ExternalOutput").ap()

    stair_np = (np.tril(np.ones((128, 128), dtype=np.float32)).T).astype(
        ml_dtypes.bfloat16
    )  # stair[i, j] = 1 if i <= j else 0
    stair_d = nc.inline_tensor(stair_np, name="stair").ap()

    with tile.TileContext(nc) as tc:
        with contextlib.ExitStack() as ctx:
            consts = ctx.enter_context(tc.tile_pool(name="consts", bufs=1))
            xpool = ctx.enter_context(tc.tile_pool(name="x", bufs=1))
            qkpool = ctx.enter_context(tc.tile_pool(name="qk", bufs=1))
            vpool = ctx.enter_context(tc.tile_pool(name="v", bufs=1))
            ppool = ctx.enter_context(tc.tile_pool(name="p", bufs=2))
            ypool = ctx.enter_context(tc.tile_pool(name="y", bufs=2))
            opool = ctx.enter_context(tc.tile_pool(name="o", bufs=3))
            rpool = ctx.enter_context(tc.tile_pool(name="r", bufs=2))
            big_ps = ctx.enter_context(tc.tile_pool(name="big_ps", bufs=2, space="PSUM"))
            s_ps = ctx.enter_context(tc.tile_pool(name="s_ps", bufs=2, space="PSUM"))
            y_ps = ctx.enter_context(tc.tile_pool(name="y_ps", bufs=2, space="PSUM"))

            # ---- constants / weights ----
            wt = {}
            for cc in range(NCC):
                wt[cc] = consts.tile([128, 3 * FPC], F32R, tag=f"w{cc}", name=f"w{cc}")
                nc.sync.dma_start(out=wt[cc][:], in_=wT_d[cc * 128:(cc + 1) * 128, :])
            wp = {}
            for hc in range(2):
                wp[hc] = consts.tile([128, C], F32R, tag=f"wp{hc}", name=f"wp{hc}")
                nc.sync.dma_start(out=wp[hc][:], in_=wpT_d[hc * 128:(hc + 1) * 128, :])
            qk_bias = {}
            for fb in range(4):
                qk_bias[fb] = consts.tile([128, 1], F32, tag=f"qkb{fb}", name=f"qkb{fb}")
                nc.sync.dma_start(
                    out=qk_bias[fb][:],
                    in_=bqkv_d[fb * 128:(fb + 1) * 128].unsqueeze(-1),
                )
            v_bias = consts.tile([128, FPC], F32, tag="vbias")
            nc.sync.dma_start(
                out=v_bias[:], in_=bqkv_d[512:768].partition_broadcast(128)
            )
            stair = consts.tile([128, 128], BF16, tag="stair")
            nc.sync.dma_start(out=stair[:], in_=stair_d[:])

            # ---- x load (token-chunk-major so compute can start early) ----
            xt = {}
            for tc_i in range(NTC):
                for cc in range(NCC):
                    xt[(cc, tc_i)] = xpool.tile([128, 512], F32R, tag=f"x{cc}_{tc_i}", name=f"x{cc}_{tc_i}")
                    nc.sync.dma_start(
                        out=xt[(cc, tc_i)][:],
                        in_=xT_d[cc * 128:(cc + 1) * 128, tc_i * 512:(tc_i + 1) * 512],
                    )

            # ---- QKV projection ----
            # feature-major Q^T, K^T: qk[(fb, tc)] [128, 512], fb 0..1 = Q
            # (heads 0-1, 2-3), fb 2..3 = K
            qk = {}
            vt_by_tb = {}
            for tc_i in range(NTC):
                for fb in range(4):
                    ps = big_ps.tile([128, 512], F32, tag="bigps")
                    for cc in range(NCC):
                        nc.tensor.matmul(
                            ps[:],
                            wt[cc][:, fb * 128:(fb + 1) * 128],
                            xt[(cc, tc_i)][:],
                            start=(cc == 0),
                            stop=(cc == NCC - 1),
                        )
                    qk[(fb, tc_i)] = qkpool.tile([128, 512], F32R, tag=f"qk{fb}_{tc_i}", name=f"qk{fb}_{tc_i}")
                    nc.vector.tensor_scalar_add(qk[(fb, tc_i)][:], ps[:], qk_bias[fb][:])
                # token-major V_ext tiles [128 tokens, 4 heads, 66] (64 V cols,
                # col 64 = ones for the softmax denominator, col 65 pad)
                for tb in range(tc_i * 4, tc_i * 4 + 4):
                    ps = big_ps.tile([128, FPC], F32, tag="bigps")
                    for cc in range(NCC):
                        nc.tensor.matmul(
                            ps[:],
                            xt[(cc, tc_i)][:, (tb % 4) * 128:(tb % 4 + 1) * 128],
                            wt[cc][:, 512:768],
                            start=(cc == 0),
                            stop=(cc == NCC - 1),
                        )
                    vt = vpool.tile([128, HPC, 66], BF16, tag=f"v{tb}")
                    nc.vector.tensor_add(
                        vt[:, :, 0:64],
                        ps[:].rearrange("p (h d) -> p h d", h=HPC),
                        v_bias[:].rearrange("p (h d) -> p h d", h=HPC),
                    )
                    nc.vector.memset(vt[:, :, 64:65], 1.0)
                    vt_by_tb[tb] = vt

            # ---- attention + proj, per query chunk ----
            for qc in range(NQC):
                yT_pair = {}
                for hc in range(2):
                    yT_pair[hc] = ypool.tile([128, 512], F32R, tag=f"yp{hc}", name=f"yp{hc}_{qc}")
                kmax = 4 * (qc + 1)
                for h in range(HPC):
                    fbq, hb = h // 2, h % 2
                    prow = slice(hb * 64, hb * 64 + 64)
                    p_pairs = {}
                    for pr in range(kmax // 2):
                        kb0 = 2 * pr
                        sp = s_ps.tile([128, 1024], F32, tag="sps")
                        for half in range(2):
                            kb = kb0 + half
                            nc.tensor.matmul(
                                sp[:, half * 512:(half + 1) * 512],
                                qk[(2 + fbq, kb // 4)][prow, (kb % 4) * 128:(kb % 4 + 1) * 128],
                                qk[(fbq, qc)][prow, :],
                                start=True,
                                stop=True,
                            )
                        pt = ppool.tile([128, 1024], BF16, tag=f"pp{pr}")
                        g0 = kb0 * 128 - qc * 512
                        if g0 <= -256:
                            nc.scalar.activation(
                                pt[:], sp[:], mybir.ActivationFunctionType.Exp,
                                scale=0.125,
                            )
                        else:
                            # diagonal pair: halves have g = g0, g0+128
                            s0 = max(g0, 0)
                            nc.scalar.activation(
                                pt[:, s0:1024], sp[:, s0:1024],
                                mybir.ActivationFunctionType.Exp, scale=0.125,
                            )
                            for half in range(2):
                                g = g0 + 128 * half
                                base = half * 512
                                if g > 0:
                                    nc.gpsimd.memset(pt[:, base:base + g], 0.0)
                                nc.vector.tensor_mul(
                                    pt[:, base + g:base + g + 127],
                                    pt[:, base + g:base + g + 127],
                                    stair[:, 0:127],
                                )
                        p_pairs[pr] = pt
                    yp = y_ps.tile([65, 512], F32, tag="yps")
                    for kb in range(kmax):
                        nc.tensor.matmul(
                            yp[:],
                            vt_by_tb[kb][:, h, 0:65],
                            p_pairs[kb // 2][:, (kb % 2) * 512:(kb % 2 + 1) * 512],
                            start=(kb == 0),
                            stop=(kb == kmax - 1),
                        )
                    y_sb = rpool.tile([65, 512], F32, tag="y_sb")
                    nc.vector.tensor_copy(y_sb[:], yp[:])
                    denom0 = rpool.tile([1, 512], F32, tag="denom0")
                    nc.sync.dma_start(out=denom0[:], in_=y_sb[64:65, :])
                    recip = rpool.tile([1, 512], F32, tag="recip")
                    nc.vector.reciprocal_approx_fast(out=recip[:], in_=denom0[:])
                    recip_b = rpool.tile([64, 512], F32, tag="recip_b")
                    nc.gpsimd.partition_broadcast(recip_b[:], recip[:])
                    nc.vector.tensor_mul(
                        yT_pair[h // 2][prow, :], y_sb[0:64, :], recip_b[:]
                    )
                # c_proj partial for this query chunk
                for ob in range(8):
                    op = big_ps.tile([128, 512], F32, tag="bigps")
                    for hc in range(2):
                        nc.tensor.matmul(
                            op[:],
                            wp[hc][:, ob * 128:(ob + 1) * 128],
                            yT_pair[hc][:],
                            start=(hc == 0),
                            stop=(hc == 1),
                        )
                    ot = opool.tile([128, 512], F32, tag="ot")
                    nc.scalar.copy(ot[:], op[:])
                    nc.sync.dma_start(
                        out=yT_d[ob * 128:(ob + 1) * 128, qc * 512:(qc + 1) * 512],
                        in_=ot[:],
                    )

    nc.compile()
    _CACHE["nc"] = nc
    return nc


def kernel(x, w_attn, b_attn, w_proj, b_proj):
    nc = _build()
    in_maps = []
    for core in range(N_CORES):
        b, s = core // 4, core % 4
        f0 = FPC * s
        xT = np.ascontiguousarray(x[b].T)
        wT = np.ascontiguousarray(
            np.concatenate(
                [
                    w_attn[f0:f0 + FPC],
                    w_attn[C + f0:C + f0 + FPC],
                    w_attn[2 * C + f0:2 * C + f0 + FPC],
                ],
                axis=0,
            ).T
        )
        bqkv = np.ascontiguousarray(
            np.concatenate(
                [
                    b_attn[f0:f0 + FPC],
                    b_attn[C + f0:C + f0 + FPC],
                    b_attn[2 * C + f0:2 * C + f0 + FPC],
                ]
            )
        )
        wpT = np.ascontiguousarray(w_proj[:, f0:f0 + FPC].T)
        in_maps.append({"xT": xT, "wT": wT, "bqkv": bqkv, "wpT": wpT})

    res = run_bass_kernel_spmd(nc, in_maps, list(range(N_CORES)))
    out = np.empty((B, T, C), dtype=np.float32)
    for b in range(B):
        acc = res.results[4 * b]["yT"].astype(np.float32)
        for s in range(1, 4):
            acc = acc + res.results[4 * b + s]["yT"]
        out[b] = acc.T + b_proj[None, :]
    return out


# revision 16
# speedup vs baseline: 1.5846x; 1.1323x over previous
"""Causal self-attention (B=2, T=2048, C=1024, H=16) on 8 TRN2 NeuronCores.

Sharding: 2 batches x 4 head-groups (4 heads each). Each core computes
qkv projection for its batch/head-slice, causal attention, and a partial
c_proj contribution; the host sums partials and adds b_proj.

Device layout (per core):
  xT  [C=1024, T=2048]  x[b] transposed (host-side)
  wT  [C=1024, 768]     w_attn rows for this head slice, transposed: cols
                        [0:256]=Q feats, [256:512]=K, [512:768]=V
  bqkv [768]            matching bias slice
  wpT [256, 1024]       w_proj columns for this head slice, transposed
  out yT [1024, 2048]   partial (c_proj output)^T, to be summed over the
                        4 cores of the batch on host

All matmuls run as float32r (full-rate fp32 on the PE, ~1.5e-4 rel err).
P = exp(scores) and V are bf16 for the attention*V matmul.
Causal structure: scores computed in S^T [key, query] orientation per
128-key-block x 512-query-chunk rectangles, only blocks intersecting the
causal triangle; diagonal blocks masked via memset + a precomputed
staircase mask. Softmax denominators come free from a ones-column
appended to V (row 64 of the AV psum accumulator).
"""
import sys
import contextlib

sys.path.insert(0, "/opt/trn_rl_repo")

import numpy as np
import ml_dtypes

import concourse.bass as bass
import concourse.mybir as mybir
import concourse.tile as tile
from concourse import bacc
from concourse.bass_utils import run_bass_kernel_spmd

B, T, C, H = 2, 2048, 1024, 16
HD = 64
N_CORES = 8
HPC = 4          # heads per core
FPC = HPC * HD   # features per core = 256
QCH = 512        # query chunk
NQC = T // QCH   # 4
NKB = T // 128   # 16 k blocks
NCC = C // 128   # 8 contraction chunks
NTC = T // 512   # 4 token chunks

F32 = mybir.dt.float32
F32R = mybir.dt.float32r
BF16 = mybir.dt.bfloat16

_CACHE: dict = {}


def _build():
    if "nc" in _CACHE:
        return _CACHE["nc"]
    nc = bacc.Bacc(None, target_bir_lowering=False, debug=False)

    xT_d = nc.dram_tensor("xT", [C, T], F32R, kind="ExternalInput").ap()
    wT_d = nc.dram_tensor("wT", [C, 3 * FPC], F32R, kind="ExternalInput").ap()
    bqkv_d = nc.dram_tensor("bqkv", [3 * FPC], F32, kind="ExternalInput").ap()
    wpT_d = nc.dram_tensor("wpT", [FPC, C], F32R, kind="ExternalInput").ap()
    yT_d = nc.dram_tensor("yT", [C, T], F32, kind="ExternalOutput").ap()

    stair_np = (np.tril(np.ones((128, 128), dtype=np.float32)).T).astype(
        ml_dtypes.bfloat16
    )  # stair[i, j] = 1 if i <= j else 0
    stair_d = nc.inline_tensor(stair_np, name="stair").ap()

    with tile.TileContext(nc) as tc:
        with contextlib.ExitStack() as ctx:
            consts = ctx.enter_context(tc.tile_pool(name="consts", bufs=1))
            xpool = ctx.enter_context(tc.tile_pool(name="x", bufs=1))
            qkpool = ctx.enter_context(tc.tile_pool(name="qk", bufs=1))
            vpool = ctx.enter_context(tc.tile_pool(name="v", bufs=1))
            ppool = ctx.enter_context(tc.tile_pool(name="p", bufs=2))
            ypool = ctx.enter_context(tc.tile_pool(name="y", bufs=2))
            opool = ctx.enter_context(tc.tile_pool(name="o", bufs=3))
            rpool = ctx.enter_context(tc.tile_pool(name="r", bufs=2))
            big_ps = ctx.enter_context(tc.tile_pool(name="big_ps", bufs=2, space="PSUM"))
            s_ps = ctx.enter_context(tc.tile_pool(name="s_ps", bufs=2, space="PSUM"))
            y_ps = ctx.enter_context(tc.tile_pool(name="y_ps", bufs=2, space="PSUM"))

            # ---- constants / weights ----
            wt = {}
            for cc in range(NCC):
                wt[cc] = consts.tile([128, 3 * FPC], F32R, tag=f"w{cc}", name=f"w{cc}")
                nc.sync.dma_start(out=wt[cc][:], in_=wT_d[cc * 128:(cc + 1) * 128, :])
            wp = {}
            for hc in range(2):
                wp[hc] = consts.tile([128, C], F32R, tag=f"wp{hc}", name=f"wp{hc}")
                nc.sync.dma_start(out=wp[hc][:], in_=wpT_d[hc * 128:(hc + 1) * 128, :])
            qk_bias = {}
            for fb in range(4):
                qk_bias[fb] = consts.tile([128, 1], F32, tag=f"qkb{fb}", name=f"qkb{fb}")
                nc.sync.dma_start(
                    out=qk_bias[fb][:],
                    in_=bqkv_d[fb * 128:(fb + 1) * 128].unsqueeze(-1),
                )
            v_bias = consts.tile([128, FPC], F32, tag="vbias")
            nc.sync.dma_start(
                out=v_bias[:], in_=bqkv_d[512:768].partition_broadcast(128)
            )
            stair = consts.tile([128, 128], BF16, tag="stair")
            nc.sync.dma_start(out=stair[:], in_=stair_d[:])

            # ---- x load (token-chunk-major so compute can start early) ----
            xt = {}
            for tc_i in range(NTC):
                for cc in range(NCC):
                    xt[(cc, tc_i)] = xpool.tile([128, 512], F32R, tag=f"x{cc}_{tc_i}", name=f"x{cc}_{tc_i}")
                    nc.sync.dma_start(
                        out=xt[(cc, tc_i)][:],
                        in_=xT_d[cc * 128:(cc + 1) * 128, tc_i * 512:(tc_i + 1) * 512],
                    )

            # ---- QKV projection ----
            # feature-major Q^T, K^T: qk[(fb, tc)] [128, 512], fb 0..1 = Q
            # (heads 0-1, 2-3), fb 2..3 = K
            qk = {}
            vt_by_tb = {}
            for tc_i in range(NTC):
                for fb in range(4):
                    ps = big_ps.tile([128, 512], F32, tag="bigps")
                    for cc in range(NCC):
                        nc.tensor.matmul(
                            ps[:],
                            wt[cc][:, fb * 128:(fb + 1) * 128],
                            xt[(cc, tc_i)][:],
                            start=(cc == 0),
                            stop=(cc == NCC - 1),
                        )
                    qk[(fb, tc_i)] = qkpool.tile([128, 512], F32R, tag=f"qk{fb}_{tc_i}", name=f"qk{fb}_{tc_i}")
                    nc.vector.tensor_scalar_add(qk[(fb, tc_i)][:], ps[:], qk_bias[fb][:])
                # token-major V_ext tiles [128 tokens, 4 heads, 66] (64 V cols,
                # col 64 = ones for the softmax denominator, col 65 pad)
                for tb in range(tc_i * 4, tc_i * 4 + 4):
                    ps = big_ps.tile([128, FPC], F32, tag="bigps")
                    for cc in range(NCC):
                        nc.tensor.matmul(
                            ps[:],
                            xt[(cc, tc_i)][:, (tb % 4) * 128:(tb % 4 + 1) * 128],
                            wt[cc][:, 512:768],
                            start=(cc == 0),
                            stop=(cc == NCC - 1),
                        )
                    vt = vpool.tile([128, HPC, 66], BF16, tag=f"v{tb}")
                    nc.vector.tensor_add(
                        vt[:, :, 0:64],
                        ps[:].rearrange("p (h d) -> p h d", h=HPC),
                        v_bias[:].rearrange("p (h d) -> p h d", h=HPC),
                    )
                    nc.vector.memset(vt[:, :, 64:65], 1.0)
                    vt_by_tb[tb] = vt

            # ---- attention + proj, per query chunk ----
            for qc in range(NQC):
                yT_pair = {}
                for hc in range(2):
                    yT_pair[hc] = ypool.tile([128, 512], F32R, tag=f"yp{hc}", name=f"yp{hc}_{qc}")
                kmax = 4 * (qc + 1)
                for hp in range(2):
                    # scores: both heads of the pair packed per k-block via
                    # row tiling (rows 0-63 / 64-127 run concurrently on PE)
                    p_tiles = {}
                    ys = {hb: y_ps.tile([65, 512], F32, tag="yps", name=f"yps{qc}_{hp}_{hb}") for hb in range(2)}
                    for kb in range(kmax):
                        sp = s_ps.tile([128, 1024], F32, tag="sps")
                        for hb in range(2):
                            rows = slice(hb * 64, hb * 64 + 64)
                            nc.tensor.matmul(
                                sp[:, hb * 512:(hb + 1) * 512],
                                qk[(2 + hp, kb // 4)][rows, (kb % 4) * 128:(kb % 4 + 1) * 128],
                                qk[(hp, qc)][rows, :],
                                start=True,
                                stop=True,
                            )
                        pt = ppool.tile([128, 1024], BF16, tag=f"p{kb % 4}", name=f"p{qc}_{hp}_{kb}")
                        g = kb * 128 - qc * 512
                        if g <= -128:
                            nc.scalar.activation(
                                pt[:], sp[:], mybir.ActivationFunctionType.Exp,
                                scale=0.125,
                            )
                        else:
                            nc.scalar.activation(
                                pt[:, g:1024], sp[:, g:1024],
                                mybir.ActivationFunctionType.Exp, scale=0.125,
                            )
                            for hb in range(2):
                                base = hb * 512
                                if g > 0:
                                    nc.gpsimd.memset(pt[:, base:base + g], 0.0)
                                nc.vector.tensor_mul(
                                    pt[:, base + g:base + g + 127],
                                    pt[:, base + g:base + g + 127],
                                    stair[:, 0:127],
                                )
                        p_tiles[kb] = pt
                        for hb in range(2):
                            nc.tensor.matmul(
                                ys[hb][:],
                                vt_by_tb[kb][:, 2 * hp + hb, 0:65],
                                pt[:, hb * 512:(hb + 1) * 512],
                                start=(kb == 0),
                                stop=(kb == kmax - 1),
                            )
                    for hb in range(2):
                        yp = ys[hb]
                        y_sb = rpool.tile([65, 512], F32, tag="y_sb")
                        nc.vector.tensor_copy(y_sb[:], yp[:])
                        denom0 = rpool.tile([1, 512], F32, tag="denom0")
                        nc.sync.dma_start(out=denom0[:], in_=y_sb[64:65, :])
                        recip = rpool.tile([1, 512], F32, tag="recip")
                        nc.vector.reciprocal_approx_fast(out=recip[:], in_=denom0[:])
                        recip_b = rpool.tile([64, 512], F32, tag="recip_b")
                        nc.gpsimd.partition_broadcast(recip_b[:], recip[:])
                        nc.vector.tensor_mul(
                            yT_pair[hp][hb * 64:(hb + 1) * 64, :], y_sb[0:64, :], recip_b[:]
                        )
                # c_proj partial for this query chunk
                for ob in range(8):
                    op = big_ps.tile([128, 512], F32, tag="bigps")
                    for hc in range(2):
                        nc.tensor.matmul(
                            op[:],
                            wp[hc][:, ob * 128:(ob + 1) * 128],
                            yT_pair[hc][:],
                            start=(hc == 0),
                            stop=(hc == 1),
                        )
                    ot = opool.tile([128, 512], F32, tag="ot")
                    nc.vector.tensor_copy(ot[:], op[:])
                    nc.sync.dma_start(
                        out=yT_d[ob * 128:(ob + 1) * 128, qc * 512:(qc + 1) * 512],
                        in_=ot[:],
                    )

    nc.compile()
    _CACHE["nc"] = nc
    return nc


def kernel(x, w_attn, b_attn, w_proj, b_proj):
    nc = _build()
    in_maps = []
    for core in range(N_CORES):
        b, s = core // 4, core % 4
        f0 = FPC * s
        xT = np.ascontiguousarray(x[b].T)
        wT = np.ascontiguousarray(
            np.concatenate(
                [
                    w_attn[f0:f0 + FPC],
                    w_attn[C + f0:C + f0 + FPC],
                    w_attn[2 * C + f0:2 * C + f0 + FPC],
                ],
                axis=0,
            ).T
        )
        bqkv = np.ascontiguousarray(
            np.concatenate(
                [
                    b_attn[f0:f0 + FPC],
                    b_attn[C + f0:C + f0 + FPC],
                    b_attn[2 * C + f0:2 * C + f0 + FPC],
                ]
            )
        )
        wpT = np.ascontiguousarray(w_proj[:, f0:f0 + FPC].T)
        in_maps.append({"xT": xT, "wT": wT, "bqkv": bqkv, "wpT": wpT})

    res = run_bass_kernel_spmd(nc, in_maps, list(range(N_CORES)))
    out = np.empty((B, T, C), dtype=np.float32)
    for b in range(B):
        acc = res.results[4 * b]["yT"].astype(np.float32)
        for s in range(1, 4):
            acc = acc + res.results[4 * b + s]["yT"]
        out[b] = acc.T + b_proj[None, :]
    return out
